# revision 14
# baseline (speedup 1.0000x reference)
"""Trainium2 Bass kernel for nn_Blur2: depthwise 4x4 blur (upfirdn2d-style,
pad=(2,1), unit stride) over input [8, 128, 256, 256] f32.

Strategy: pure data parallel over the 1024 independent (n, c) planes --
128 planes per NeuronCore. Within a plane the 2D 16-tap conv runs on the
tensor engine as banded matmuls: the H-direction conv is the contraction
(banded Toeplitz fp16 weights, image rows on partitions) and the
W-direction conv is 4 shifted slices of the moving operand accumulated
into the same PSUM tile (clipped column ranges encode the zero padding,
clipped weight bands encode the H padding).

Precision: single fp16 cast of the input (tolerance is 2e-2; fp16-only
input + fp16 output quantization give ~5e-4 max rel err). Blur weights
(entries k/16) are exact in fp16; fp16 products accumulate exactly in
fp32 PSUM. The fp16 DRAM output is upcast to f32 on the host.

Engine balance: the blur kernel is separable (rank-1), so for half the
planes the W-direction conv is offloaded from the PE to the vector
engine: PE does a single H-only banded matmul into a per-plane PSUM
strip (with zeroed 2/1-col borders so the 4 W-tap reads are uniform),
then 3 fused scalar_tensor_tensor ops on DVE apply the W taps
(w0*y0+w1*y1) and (w3*y3+w2*y2) then combine, writing the fp16 output
tile directly. This cuts PE time from ~122 us to ~78 us, pushing the
kernel onto the ~34 MB DMA roofline (~105 us).

DMA: planes are packed in OCTS on host -- each DRAM row holds 8 planes'
fp16 data = 4 KB -- so every DMA moves >=4KB per partition (the
difference between ~100 GB/s and ~340 GB/s per core on TRN2). The output
(also 8 planes x fp16 = 4KB rows) uses a 260-row-per-oct DRAM layout
(junk rows at 127, 253..255) so both store DMAs are exactly 128
partitions: the HWDGE splits an SBUF->DRAM DMA across its 16 SDMA
engines only when the partition count divides into 16 chunks. Loads go
on the sync HWDGE ring, stores on the scalar HWDGE ring.

Per core: ~1076 matmuls x ~109 ns PE floor ~= 117 us, ~34 MB DMA.
"""
import sys

for _p in ("/opt/trn_rl_repo", "/opt/pypackages"):
    if _p not in sys.path:
        sys.path.insert(0, _p)

import contextlib

import numpy as np


def _install_ntff_hook_shim():
    """The agent image's antenv lacks axon_hooks, which bass_utils needs
    for trace=True under axon. Provide it in sys.modules, backed by
    trn_agent_boot's ctypes NTFF shim."""
    import types

    if "antenv.axon_hooks" in sys.modules:
        return
    mod = types.ModuleType("antenv.axon_hooks")
    state = {"hook": None, "tried": False}

    def set_axon_ntff_profile_hook(hook):
        state["hook"] = hook

    def get_axon_ntff_profile_hook():
        if state["hook"] is None and not state["tried"]:
            state["tried"] = True
            try:
                from trn_agent_boot.trn_boot import _ntff_profile_via_ctypes

                state["hook"] = _ntff_profile_via_ctypes("/opt/axon/libaxon_pjrt.so")
            except Exception:
                state["hook"] = None
        return state["hook"]

    mod.set_axon_ntff_profile_hook = set_axon_ntff_profile_hook
    mod.get_axon_ntff_profile_hook = get_axon_ntff_profile_hook
    sys.modules["antenv.axon_hooks"] = mod
    try:
        import antenv

        antenv.axon_hooks = mod
    except ImportError:
        pass


_install_ntff_hook_shim()

import concourse.bacc as bacc
import concourse.tile as tile
from concourse import mybir
from concourse.bass_utils import run_bass_kernel_spmd

N_CORES = 8
H = W = 256
PLANES = 1024 // N_CORES  # 128 per core
O = 8  # planes packed per SBUF/DRAM row (4KB fp16)
NOCT = PLANES // O  # 16 oct-groups per core
QP = 4  # planes per PSUM tile (psum tile [128, QP*W] f32 = 2 banks)

# M-tile layout along H per plane:
#   tile A: out rows [0, 127)   from x rows [0, 128)
#   tile B: out rows [127, 252) from x rows [125, 253)
#   remainder: out rows [252, 256) from x rows [250, 256), stacked across
#   groups of RG=16 octs (96 partitions, 4 out rows per plane-slot)
MA, MB = 127, 125
RG = 16

# per W-shift i: out cols [wl, wh), reading x cols [cl, ch)  (tap = w-2+i)
SHIFT_RANGES = {
    0: (2, 256, 0, 254),
    1: (1, 256, 0, 255),
    2: (0, 256, 0, 256),
    3: (0, 255, 1, 256),
}
SHIFT_ORDER = [2, 0, 1, 3]  # full-range shift first so start=True covers all


def _separable(wk: np.ndarray):
    """Rank-1 factorization wk = outer(uh, uw); returns (uh, uw) or None."""
    u, s, vt = np.linalg.svd(wk.astype(np.float64))
    if s[1] > 1e-6 * s[0]:
        return None
    uh = u[:, 0] * np.sqrt(s[0])
    uw = vt[0] * np.sqrt(s[0])
    if uh.sum() < 0:
        uh, uw = -uh, -uw
    if abs(uw[1]) < 1e-12 or abs(uw[2]) < 1e-12:
        return None
    return uh, uw


def _make_weights(wk: np.ndarray):
    """wk: flipped 4x4 kernel. Packed fp16 weights, one 128-col matrix per
    W-shift (cols padded with zeros past MA/MB so NumWeights==128 enables
    the PE Fast-Weight-Load path): wa/wb [128, 4*128], wr [96, 4*64]
    (block-diag 16x(6->4)). Plus H-only banded mats wa2/wb2 [128, 128]
    (taps uh*uw[2]) for the DVE W-conv offload path."""
    sep = _separable(wk)
    if sep is not None:
        uh, uw = sep
        uh2 = (uh * uw[2]).astype(np.float32)
        wa2 = np.zeros((128, 128), np.float32)
        wb2 = np.zeros((128, 128), np.float32)
        for k in range(128):
            for m in range(MA):
                d = k - m + 2
                if 0 <= d <= 3:
                    wa2[k, m] = uh2[d]
            for m in range(MB):
                d = k - m
                if 0 <= d <= 3:
                    wb2[k, m] = uh2[d]
    else:
        wa2 = np.zeros((128, 128), np.float32)
        wb2 = np.zeros((128, 128), np.float32)
    wa = np.zeros((128, 4, 128), np.float32)
    for k in range(128):
        for m in range(MA):
            d = k - m + 2
            if 0 <= d <= 3:
                wa[k, :, m] = wk[d, :]
    wb = np.zeros((128, 4, 128), np.float32)
    for k in range(128):
        for m in range(MB):
            d = k - m
            if 0 <= d <= 3:
                wb[k, :, m] = wk[d, :]
    wr = np.zeros((RG * 6, 4, RG * 4), np.float32)
    for b in range(RG):
        for r in range(6):
            for c in range(4):
                d = r - c
                if 0 <= d <= 3:
                    wr[6 * b + r, :, 4 * b + c] = wk[d, :]
    return (
        wa.reshape(128, 4 * 128).astype(np.float16),
        wb.reshape(128, 4 * 128).astype(np.float16),
        wr.reshape(RG * 6, 4 * RG * 4).astype(np.float16),
        wa2.astype(np.float16),
        wb2.astype(np.float16),
    )


def _build_program(noct: int = NOCT, ratios=None):
    nc = bacc.Bacc("TRN2", target_bir_lowering=False, debug=False)
    f16, f32 = mybir.dt.float16, mybir.dt.float32
    offload = ratios is not None

    d_xs = nc.dram_tensor("xs", [noct, H, O * W], f16, kind="ExternalInput").ap()
    d_wa = nc.dram_tensor("wa", [128, 4 * 128], f16, kind="ExternalInput").ap()
    d_wb = nc.dram_tensor("wb", [128, 4 * 128], f16, kind="ExternalInput").ap()
    d_wr = nc.dram_tensor("wr", [RG * 6, 4 * RG * 4], f16, kind="ExternalInput").ap()
    d_wa2 = nc.dram_tensor("wa2", [128, 128], f16, kind="ExternalInput").ap()
    d_wb2 = nc.dram_tensor("wb2", [128, 128], f16, kind="ExternalInput").ap()
    d_out = nc.dram_tensor("out", [noct, H + 4, O * W], f16, kind="ExternalOutput").ap()

    rem_groups = [(s, min(RG, noct - s)) for s in range(0, noct, RG)]

    with tile.TileContext(nc) as tc, contextlib.ExitStack() as ctx:
        wpool = ctx.enter_context(tc.tile_pool(name="wpool", bufs=1))
        xin = ctx.enter_context(tc.tile_pool(name="xin", bufs=6))
        xinr = ctx.enter_context(tc.tile_pool(name="xinr", bufs=2))
        psum = ctx.enter_context(tc.tile_pool(name="psum", bufs=2, space="PSUM"))
        psyp = ctx.enter_context(tc.tile_pool(name="psyp", bufs=4, space="PSUM"))  # 4 x [128,256] f32 = 2 banks
        scr = ctx.enter_context(tc.tile_pool(name="scr", bufs=3))
        outp = ctx.enter_context(tc.tile_pool(name="outp", bufs=4))
        outr = ctx.enter_context(tc.tile_pool(name="outr", bufs=2))

        # PE warmup: ~20 junk matmuls with no data dependencies, issued
        # before any real work. They run during the DMA ramp (t~5-10us)
        # and lift the HAM clock gate to 2.4 GHz before the real stream
        # starts. Results land in a scratch psum slot and are discarded;
        # any garbage/NaN is overwritten later because every bank's first
        # real matmul runs with start=True.
        warm = wpool.tile([128, W], f16, tag="warm")
        nc.vector.memset(warm[:], 0.0)
        psW = psum.tile([128, W], f32, tag="psA")
        for _ in range(20):
            nc.tensor.matmul(
                psW[:, :], warm[:, :128], warm[:, :],
                start=True, stop=True, skip_group_check=True,
            )

        t_wa = wpool.tile([128, 4 * 128], f16, tag="wa")
        nc.scalar.dma_start(out=t_wa[:], in_=d_wa)
        t_wb = wpool.tile([128, 4 * 128], f16, tag="wb")
        nc.scalar.dma_start(out=t_wb[:], in_=d_wb)
        t_wr = wpool.tile([RG * 6, 4 * RG * 4], f16, tag="wr")
        nc.scalar.dma_start(out=t_wr[:], in_=d_wr)
        t_wa2 = wpool.tile([128, 128], f16, tag="wa2")
        nc.scalar.dma_start(out=t_wa2[:], in_=d_wa2)
        t_wb2 = wpool.tile([128, 128], f16, tag="wb2")
        nc.scalar.dma_start(out=t_wb2[:], in_=d_wb2)

        def conv_mms(ps, wt, xt, xrows, qbase):
            """4 shifts x QP planes accumulating matmuls into the psum tile
            ps [128, QP*W] (2 banks; per-bank first mm gets start=True).
            Planes qbase..qbase+QP of the oct input tile xt."""
            last = (SHIFT_ORDER[-1], QP - 1)
            for i in SHIFT_ORDER:
                wl, wh, cl, ch = SHIFT_RANGES[i]
                lhsT = wt[:xrows, i * 128 : i * 128 + 128]
                for q in range(QP):
                    nc.tensor.matmul(
                        ps[:128, q * W + wl : q * W + wh],
                        lhsT,
                        xt[:xrows, (qbase + q) * W + cl : (qbase + q) * W + ch],
                        start=(i == SHIFT_ORDER[0] and q % 2 == 0),
                        stop=((i, q) == last),
                        skip_group_check=True,
                    )

        def copy_half(o, ps, lo, alt):
            """psum [128, QP*W] f32 -> fp16 out tile columns [lo, lo+QP*W),
            split between scalar and vector engines."""
            hw = QP * W // 2
            if alt:
                nc.scalar.copy(o[:, lo : lo + hw], ps[:, :hw])
                nc.vector.tensor_copy(o[:, lo + hw : lo + 2 * hw], ps[:, hw:])
            else:
                nc.vector.tensor_copy(o[:, lo : lo + hw], ps[:, :hw])
                nc.scalar.copy(o[:, lo + hw : lo + 2 * hw], ps[:, hw:])

        # Static y-staging strips for the offload path: borders (W zero-pad)
        # are memset ONCE here and never rewritten -- the per-plane stage
        # copy only fills cols [2, 258), so tap reads strip[:, i:i+256] see
        # zeros at the pad positions. 4 strips rotated manually give the
        # same pipelining as a pool without per-use border memsets.
        strips = []
        if offload:
            for si in range(4):
                st = wpool.tile([128, 260], f32, tag=f"st{si}")
                nc.vector.memset(st[:, 0:2], 0.0)
                nc.vector.memset(st[:, 258:260], 0.0)
                strips.append(st)
        pc = [0]

        def offload_plane(o, wt2, xt, p):
            """Separable path for plane p of the oct: PE computes the H-only
            conv y (scaled by uw2) into a half-bank PSUM tile; y is staged
            to an SBUF strip (DVE can read at most one PSUM operand per op),
            then 3 DVE scalar_tensor_tensor ops apply the W taps and write
            the fp16 output tile directly."""
            r01, r32, r12 = ratios
            mult, add = mybir.AluOpType.mult, mybir.AluOpType.add
            strip = strips[pc[0] % 4]
            stage_scalar = pc[0] % 2 == 0
            pc[0] += 1
            psy = psyp.tile([128, 256], f32, tag="psy")
            nc.tensor.matmul(
                psy[:128, 0:256], wt2[:128, :], xt[:128, p * W : (p + 1) * W],
                start=True, stop=True, skip_group_check=True,
            )
            if stage_scalar:
                nc.scalar.copy(strip[:, 2:258], psy[:, 0:256])
            else:
                nc.vector.tensor_copy(strip[:, 2:258], psy[:, 0:256])
            sc = scr.tile([128, 512], f32, tag="sc")
            nc.vector.scalar_tensor_tensor(
                out=sc[:, 0:256], in0=strip[:, 0:256], scalar=r01,
                in1=strip[:, 1:257], op0=mult, op1=add,
            )
            nc.vector.scalar_tensor_tensor(
                out=sc[:, 256:512], in0=strip[:, 3:259], scalar=r32,
                in1=strip[:, 2:258], op0=mult, op1=add,
            )
            nc.vector.scalar_tensor_tensor(
                out=o[:, p * W : (p + 1) * W], in0=sc[:, 0:256], scalar=r12,
                in1=sc[:, 256:512], op0=mult, op1=add,
            )

        ri = 0
        for g in range(noct):
            ta = xin.tile([128, O * W], f16, tag="ta")
            if g == 0:
                # split the very first load so the PE stream starts sooner
                nc.sync.dma_start(out=ta[0:64, :], in_=d_xs[g, 0:64, :])
                nc.sync.dma_start(out=ta[64:128, :], in_=d_xs[g, 64:128, :])
            else:
                nc.sync.dma_start(out=ta[:], in_=d_xs[g, 0:128, :])
            tb = xin.tile([128, O * W], f16, tag="tb")
            nc.sync.dma_start(out=tb[:], in_=d_xs[g, 125:253, :])

            oa = outp.tile([128, O * W], f16, tag="oa")
            if offload:
                for p in range(QP, O):
                    offload_plane(oa, t_wa2, ta, p)
                psA = psum.tile([128, QP * W], f32, tag="psA")
                conv_mms(psA, t_wa, ta, 128, 0)
                nc.scalar.copy(oa[:, 0 : QP * W], psA[:, :])
            else:
                for h in range(O // QP):
                    psA = psum.tile([128, QP * W], f32, tag="psA")
                    conv_mms(psA, t_wa, ta, 128, h * QP)
                    copy_half(oa, psA, h * QP * W, alt=((g + h) % 2 == 0))
            nc.scalar.dma_start(out=d_out[g, 0:128, :], in_=oa[:])

            ob = outp.tile([128, O * W], f16, tag="ob")
            if offload:
                for p in range(QP, O):
                    offload_plane(ob, t_wb2, tb, p)
                psB = psum.tile([128, QP * W], f32, tag="psA")
                conv_mms(psB, t_wb, tb, 128, 0)
                nc.scalar.copy(ob[:, 0 : QP * W], psB[:, :])
            else:
                for h in range(O // QP):
                    psB = psum.tile([128, QP * W], f32, tag="psA")
                    conv_mms(psB, t_wb, tb, 128, h * QP)
                    copy_half(ob, psB, h * QP * W, alt=((g + h) % 2 == 1))
            nc.scalar.dma_start(out=d_out[g, 128:256, :], in_=ob[:])

            # stacked remainder: input rows come straight from DRAM, so
            # emit early (oct 2, 4, ...) to keep them off the kernel tail
            if ri < len(rem_groups) and g == min(2 * (ri + 1), noct - 1):
                s, gsz = rem_groups[ri]
                ri += 1
                tr = xinr.tile([RG * 6, O * W], f16, tag="tr")
                nc.sync.dma_start(
                    out=tr[: 6 * gsz, :], in_=d_xs[s : s + gsz, 250:256, :]
                )
                orr = outr.tile([RG * 4, O * W], f16, tag="orr")
                for h in range(O // QP):
                    psR = psum.tile([RG * 4, QP * W], f32, tag="psA")
                    last = (SHIFT_ORDER[-1], QP - 1)
                    for i in SHIFT_ORDER:
                        wl, wh, cl, ch = SHIFT_RANGES[i]
                        lhsT = t_wr[: 6 * gsz, i * RG * 4 : i * RG * 4 + 4 * gsz]
                        for q in range(QP):
                            nc.tensor.matmul(
                                psR[: 4 * gsz, q * W + wl : q * W + wh],
                                lhsT,
                                tr[: 6 * gsz, (h * QP + q) * W + cl : (h * QP + q) * W + ch],
                                start=(i == SHIFT_ORDER[0] and q % 2 == 0),
                                stop=((i, q) == last),
                                skip_group_check=True,
                            )
                    if (g + h) % 2 == 0:
                        nc.scalar.copy(
                            orr[: 4 * gsz, h * QP * W : (h + 1) * QP * W],
                            psR[: 4 * gsz, :],
                        )
                    else:
                        nc.vector.tensor_copy(
                            orr[: 4 * gsz, h * QP * W : (h + 1) * QP * W],
                            psR[: 4 * gsz, :],
                        )
                nc.scalar.dma_start(
                    out=d_out[s : s + gsz, H : H + 4, :], in_=orr[: 4 * gsz, :]
                )

    nc.compile()
    return nc


_CACHE = {}


def _get_program(noct: int = NOCT, ratios=None):
    key = (noct, ratios)
    if key not in _CACHE:
        _CACHE[key] = _build_program(noct, ratios)
    return _CACHE[key]


def _run(x: np.ndarray, wk: np.ndarray, trace: bool = False):
    """x: [P, 256, 256] f32 full stack of planes (P divisible by 8*O),
    wk: flipped 4x4 kernel. Returns ([P, 256, 256] f32, exec_time_ns|None)."""
    P = x.shape[0]
    oper = P // (N_CORES * O)
    hi = x.astype(np.float16)
    # oct-pack: [P/O, O, H, W] -> [P/O, H, O, W] -> [P/O, H, O*W]
    xso = (
        hi.reshape(P // O, O, H, W)
        .transpose(0, 2, 1, 3)
        .reshape(P // O, H, O * W)
    )

    wa, wb, wr, wa2, wb2 = _make_weights(wk)
    sep = _separable(wk)
    ratios = None
    if sep is not None:
        uh, uw = sep
        ratios = (
            float(uw[0] / uw[1]),
            float(uw[3] / uw[2]),
            float(uw[1] / uw[2]),
        )
    nc = _get_program(oper, ratios)

    in_maps = [
        {
            "xs": np.ascontiguousarray(xso[c * oper : (c + 1) * oper]),
            "wa": wa,
            "wb": wb,
            "wr": wr,
            "wa2": wa2,
            "wb2": wb2,
        }
        for c in range(N_CORES)
    ]
    res = run_bass_kernel_spmd(nc, in_maps, list(range(N_CORES)), trace=trace)
    outq = np.concatenate([r["out"] for r in res.results], axis=0)  # [P/O, H+4, O*W]
    outq = np.concatenate(
        [outq[:, 0:127], outq[:, 128:253], outq[:, 256:260]], axis=1
    )  # drop junk rows -> [P/O, 256, O*W]
    out = (
        outq.reshape(P // O, H, O, W)
        .transpose(0, 2, 1, 3)
        .reshape(P, H, W)
        .astype(np.float32)
    )
    return np.ascontiguousarray(out), res.exec_time_ns


def kernel(input: np.ndarray, kernel: np.ndarray) -> np.ndarray:
    x = np.asarray(input, dtype=np.float32)
    k = np.asarray(kernel, dtype=np.float32)
    n, c, h, w = x.shape
    wk = np.flip(k, (0, 1)).copy()  # correlation weights
    out, _ = _run(x.reshape(n * c, h, w), wk, trace=False)
    return out.reshape(n, c, h, w)


# revision 17
# speedup vs baseline: 1.3740x; 1.3740x over previous
"""Trainium2 Bass kernel for nn_Blur2: depthwise 4x4 blur (upfirdn2d-style,
pad=(2,1), unit stride) over input [8, 128, 256, 256] f32.

Strategy: pure data parallel over the 1024 independent (n, c) planes --
128 planes per NeuronCore. Within a plane the 2D 16-tap conv runs on the
tensor engine as banded matmuls: the H-direction conv is the contraction
(banded Toeplitz fp16 weights, image rows on partitions) and the
W-direction conv is 4 shifted slices of the moving operand accumulated
into the same PSUM tile (clipped column ranges encode the zero padding,
clipped weight bands encode the H padding).

Precision: single fp16 cast of the input (tolerance is 2e-2; fp16-only
input + fp16 output quantization give ~5e-4 max rel err). Blur weights
(entries k/16) are exact in fp16; fp16 products accumulate exactly in
fp32 PSUM. The fp16 DRAM output is upcast to f32 on the host.

Engine balance: the blur kernel is separable (rank-1), so for half the
planes the W-direction conv is offloaded from the PE to the vector
engine: PE does a single H-only banded matmul into a per-plane PSUM
strip (with zeroed 2/1-col borders so the 4 W-tap reads are uniform),
then 3 fused scalar_tensor_tensor ops on DVE apply the W taps
(w0*y0+w1*y1) and (w3*y3+w2*y2) then combine, writing the fp16 output
tile directly. This cuts PE time from ~122 us to ~78 us, pushing the
kernel onto the ~34 MB DMA roofline (~105 us).

DMA: planes are packed in OCTS on host -- each DRAM row holds 8 planes'
fp16 data = 4 KB -- so every DMA moves >=4KB per partition (the
difference between ~100 GB/s and ~340 GB/s per core on TRN2). The output
(also 8 planes x fp16 = 4KB rows) uses a 260-row-per-oct DRAM layout
(junk rows at 127, 253..255) so both store DMAs are exactly 128
partitions: the HWDGE splits an SBUF->DRAM DMA across its 16 SDMA
engines only when the partition count divides into 16 chunks. Loads go
on the sync HWDGE ring, stores on the scalar HWDGE ring.

Per core: ~1076 matmuls x ~109 ns PE floor ~= 117 us, ~34 MB DMA.
"""
import sys

for _p in ("/opt/trn_rl_repo", "/opt/pypackages"):
    if _p not in sys.path:
        sys.path.insert(0, _p)

import contextlib

import numpy as np


def _install_ntff_hook_shim():
    """The agent image's antenv lacks axon_hooks, which bass_utils needs
    for trace=True under axon. Provide it in sys.modules, backed by
    trn_agent_boot's ctypes NTFF shim."""
    import types

    if "antenv.axon_hooks" in sys.modules:
        return
    mod = types.ModuleType("antenv.axon_hooks")
    state = {"hook": None, "tried": False}

    def set_axon_ntff_profile_hook(hook):
        state["hook"] = hook

    def get_axon_ntff_profile_hook():
        if state["hook"] is None and not state["tried"]:
            state["tried"] = True
            try:
                from trn_agent_boot.trn_boot import _ntff_profile_via_ctypes

                state["hook"] = _ntff_profile_via_ctypes("/opt/axon/libaxon_pjrt.so")
            except Exception:
                state["hook"] = None
        return state["hook"]

    mod.set_axon_ntff_profile_hook = set_axon_ntff_profile_hook
    mod.get_axon_ntff_profile_hook = get_axon_ntff_profile_hook
    sys.modules["antenv.axon_hooks"] = mod
    try:
        import antenv

        antenv.axon_hooks = mod
    except ImportError:
        pass


_install_ntff_hook_shim()

import concourse.bacc as bacc
import concourse.tile as tile
from concourse import mybir
from concourse.bass_utils import run_bass_kernel_spmd

N_CORES = 8
H = W = 256
PLANES = 1024 // N_CORES  # 128 per core
O = 8  # planes packed per SBUF/DRAM row (4KB fp16)
NOCT = PLANES // O  # 16 oct-groups per core
QP = 4  # planes per PSUM tile in the non-separable fallback path
NOFF = 2  # planes per tier offloaded to the DVE W-conv path (6 stay fused
          # on the PE so it remains ~88% busy -- the HAM clock governor
          # halves the core clock whenever PE activity drops for a ~3.4us
          # window, which slows every engine; see the 176us regression)
NF = O - NOFF  # fused planes per tier

# M-tile layout along H per plane:
#   tile A: out rows [0, 127)   from x rows [0, 128)
#   tile B: out rows [127, 252) from x rows [125, 253)
#   remainder: out rows [252, 256) from x rows [250, 256), stacked across
#   groups of RG=16 octs (96 partitions, 4 out rows per plane-slot)
MA, MB = 127, 125
RG = 16

# per W-shift i: out cols [wl, wh), reading x cols [cl, ch)  (tap = w-2+i)
SHIFT_RANGES = {
    0: (2, 256, 0, 254),
    1: (1, 256, 0, 255),
    2: (0, 256, 0, 256),
    3: (0, 255, 1, 256),
}
SHIFT_ORDER = [2, 0, 1, 3]  # full-range shift first so start=True covers all


def _separable(wk: np.ndarray):
    """Rank-1 factorization wk = outer(uh, uw); returns (uh, uw) or None."""
    u, s, vt = np.linalg.svd(wk.astype(np.float64))
    if s[1] > 1e-6 * s[0]:
        return None
    uh = u[:, 0] * np.sqrt(s[0])
    uw = vt[0] * np.sqrt(s[0])
    if uh.sum() < 0:
        uh, uw = -uh, -uw
    if abs(uw[1]) < 1e-12 or abs(uw[2]) < 1e-12:
        return None
    return uh, uw


def _make_weights(wk: np.ndarray):
    """wk: flipped 4x4 kernel. Packed fp16 weights, one 128-col matrix per
    W-shift (cols padded with zeros past MA/MB so NumWeights==128 enables
    the PE Fast-Weight-Load path): wa/wb [128, 4*128], wr [96, 4*64]
    (block-diag 16x(6->4)). Plus H-only banded mats wa2/wb2 [128, 128]
    (taps uh*uw[2]) for the DVE W-conv offload path."""
    sep = _separable(wk)
    if sep is not None:
        uh, uw = sep
        uh2 = (uh * uw[2]).astype(np.float32)
        wa2 = np.zeros((128, 128), np.float32)
        wb2 = np.zeros((128, 128), np.float32)
        for k in range(128):
            for m in range(MA):
                d = k - m + 2
                if 0 <= d <= 3:
                    wa2[k, m] = uh2[d]
            for m in range(MB):
                d = k - m
                if 0 <= d <= 3:
                    wb2[k, m] = uh2[d]
    else:
        wa2 = np.zeros((128, 128), np.float32)
        wb2 = np.zeros((128, 128), np.float32)
    wa = np.zeros((128, 4, 128), np.float32)
    for k in range(128):
        for m in range(MA):
            d = k - m + 2
            if 0 <= d <= 3:
                wa[k, :, m] = wk[d, :]
    wb = np.zeros((128, 4, 128), np.float32)
    for k in range(128):
        for m in range(MB):
            d = k - m
            if 0 <= d <= 3:
                wb[k, :, m] = wk[d, :]
    wr = np.zeros((RG * 6, 4, RG * 4), np.float32)
    for b in range(RG):
        for r in range(6):
            for c in range(4):
                d = r - c
                if 0 <= d <= 3:
                    wr[6 * b + r, :, 4 * b + c] = wk[d, :]
    return (
        wa.reshape(128, 4 * 128).astype(np.float16),
        wb.reshape(128, 4 * 128).astype(np.float16),
        wr.reshape(RG * 6, 4 * RG * 4).astype(np.float16),
        wa2.astype(np.float16),
        wb2.astype(np.float16),
    )


def _build_program(noct: int = NOCT, ratios=None):
    nc = bacc.Bacc("TRN2", target_bir_lowering=False, debug=False)
    f16, f32 = mybir.dt.float16, mybir.dt.float32
    offload = ratios is not None

    d_xs = nc.dram_tensor("xs", [noct, H, O * W], f16, kind="ExternalInput").ap()
    d_wa = nc.dram_tensor("wa", [128, 4 * 128], f16, kind="ExternalInput").ap()
    d_wb = nc.dram_tensor("wb", [128, 4 * 128], f16, kind="ExternalInput").ap()
    d_wr = nc.dram_tensor("wr", [RG * 6, 4 * RG * 4], f16, kind="ExternalInput").ap()
    d_wa2 = nc.dram_tensor("wa2", [128, 128], f16, kind="ExternalInput").ap()
    d_wb2 = nc.dram_tensor("wb2", [128, 128], f16, kind="ExternalInput").ap()
    d_out = nc.dram_tensor("out", [noct, H + 4, O * W], f16, kind="ExternalOutput").ap()

    rem_groups = [(s, min(RG, noct - s)) for s in range(0, noct, RG)]

    with tile.TileContext(nc) as tc, contextlib.ExitStack() as ctx:
        wpool = ctx.enter_context(tc.tile_pool(name="wpool", bufs=1))
        xin = ctx.enter_context(tc.tile_pool(name="xin", bufs=6))
        xinr = ctx.enter_context(tc.tile_pool(name="xinr", bufs=2))
        psum = ctx.enter_context(tc.tile_pool(name="psum", bufs=2, space="PSUM"))
        psyp = ctx.enter_context(tc.tile_pool(name="psyp", bufs=2, space="PSUM"))  # 2 x [128,256] f32 (bank-rounded) = 2 banks
        scr = ctx.enter_context(tc.tile_pool(name="scr", bufs=3))
        outp = ctx.enter_context(tc.tile_pool(name="outp", bufs=4))
        outr = ctx.enter_context(tc.tile_pool(name="outr", bufs=2))

        # PE warmup: ~20 junk matmuls with no data dependencies, issued
        # before any real work. They run during the DMA ramp (t~5-10us)
        # and lift the HAM clock gate to 2.4 GHz before the real stream
        # starts. Results land in a scratch psum slot and are discarded;
        # any garbage/NaN is overwritten later because every bank's first
        # real matmul runs with start=True.
        warm = wpool.tile([128, W], f16, tag="warm")
        nc.vector.memset(warm[:], 0.0)
        psW = psum.tile([128, W], f32, tag="psA")
        for _ in range(20):
            nc.tensor.matmul(
                psW[:, :], warm[:, :128], warm[:, :],
                start=True, stop=True, skip_group_check=True,
            )

        t_wa = wpool.tile([128, 4 * 128], f16, tag="wa")
        nc.scalar.dma_start(out=t_wa[:], in_=d_wa)
        t_wb = wpool.tile([128, 4 * 128], f16, tag="wb")
        nc.scalar.dma_start(out=t_wb[:], in_=d_wb)
        t_wr = wpool.tile([RG * 6, 4 * RG * 4], f16, tag="wr")
        nc.scalar.dma_start(out=t_wr[:], in_=d_wr)
        t_wa2 = wpool.tile([128, 128], f16, tag="wa2")
        nc.scalar.dma_start(out=t_wa2[:], in_=d_wa2)
        t_wb2 = wpool.tile([128, 128], f16, tag="wb2")
        nc.scalar.dma_start(out=t_wb2[:], in_=d_wb2)

        def conv_mms(ps, wt, xt, xrows, qbase, npl=QP):
            """4 shifts x npl planes accumulating matmuls into the psum tile
            ps [128, npl*W] (per-bank first mm gets start=True).
            Planes qbase..qbase+npl of the oct input tile xt."""
            last = (SHIFT_ORDER[-1], npl - 1)
            for i in SHIFT_ORDER:
                wl, wh, cl, ch = SHIFT_RANGES[i]
                lhsT = wt[:xrows, i * 128 : i * 128 + 128]
                for q in range(npl):
                    nc.tensor.matmul(
                        ps[:128, q * W + wl : q * W + wh],
                        lhsT,
                        xt[:xrows, (qbase + q) * W + cl : (qbase + q) * W + ch],
                        start=(i == SHIFT_ORDER[0] and q % 2 == 0),
                        stop=((i, q) == last),
                        skip_group_check=True,
                    )

        def copy_half(o, ps, lo, alt, npl=QP):
            """psum [128, npl*W] f32 -> fp16 out tile columns [lo, lo+npl*W),
            split between scalar and vector engines."""
            hw = npl * W // 2
            if alt:
                nc.scalar.copy(o[:, lo : lo + hw], ps[:, :hw])
                nc.vector.tensor_copy(o[:, lo + hw : lo + 2 * hw], ps[:, hw:])
            else:
                nc.vector.tensor_copy(o[:, lo : lo + hw], ps[:, :hw])
                nc.scalar.copy(o[:, lo + hw : lo + 2 * hw], ps[:, hw:])

        # Static y-staging strips for the offload path: borders (W zero-pad)
        # are memset ONCE here and never rewritten -- the per-plane stage
        # copy only fills cols [2, 258), so tap reads strip[:, i:i+256] see
        # zeros at the pad positions. 4 strips rotated manually give the
        # same pipelining as a pool without per-use border memsets.
        strips = []
        if offload:
            for si in range(4):
                st = wpool.tile([128, 260], f16, tag=f"st{si}")
                nc.vector.memset(st[:, 0:2], 0.0)
                nc.vector.memset(st[:, 258:260], 0.0)
                strips.append(st)
        pc = [0]

        def offload_plane(o, wt2, xt, p):
            """Separable path for plane p of the oct: PE computes the H-only
            conv y (scaled by uw2) into a half-bank PSUM tile; y is staged
            to an SBUF strip (DVE can read at most one PSUM operand per op),
            then 3 DVE scalar_tensor_tensor ops apply the W taps and write
            the fp16 output tile directly."""
            r01, r32, r12 = ratios
            mult, add = mybir.AluOpType.mult, mybir.AluOpType.add
            strip = strips[pc[0] % 4]
            pc[0] += 1
            psy = psyp.tile([128, 256], f32, tag="psy")
            nc.tensor.matmul(
                psy[:128, 0:256], wt2[:128, :], xt[:128, p * W : (p + 1) * W],
                start=True, stop=True, skip_group_check=True,
            )
            # stage y to SBUF as fp16 on the scalar engine: DVE can read at
            # most one PSUM operand per op, and 16-bit operands double its
            # throughput (f32 runs at ~123 G elem/s, fp16 at ~245)
            nc.scalar.copy(strip[:, 2:258], psy[:, 0:256])
            sc = scr.tile([128, 512], f16, tag="sc")
            nc.vector.scalar_tensor_tensor(
                out=sc[:, 0:256], in0=strip[:, 0:256], scalar=r01,
                in1=strip[:, 1:257], op0=mult, op1=add,
            )
            nc.vector.scalar_tensor_tensor(
                out=sc[:, 256:512], in0=strip[:, 3:259], scalar=r32,
                in1=strip[:, 2:258], op0=mult, op1=add,
            )
            nc.vector.scalar_tensor_tensor(
                out=o[:, p * W : (p + 1) * W], in0=sc[:, 0:256], scalar=r12,
                in1=sc[:, 256:512], op0=mult, op1=add,
            )

        ri = 0
        for g in range(noct):
            ta = xin.tile([128, O * W], f16, tag="ta")
            if g == 0:
                # split the very first load so the PE stream starts sooner
                nc.sync.dma_start(out=ta[0:64, :], in_=d_xs[g, 0:64, :])
                nc.sync.dma_start(out=ta[64:128, :], in_=d_xs[g, 64:128, :])
            else:
                nc.sync.dma_start(out=ta[:], in_=d_xs[g, 0:128, :])
            tb = xin.tile([128, O * W], f16, tag="tb")
            nc.sync.dma_start(out=tb[:], in_=d_xs[g, 125:253, :])

            oa = outp.tile([128, O * W], f16, tag="oa")
            if offload:
                for p in range(NF, O):
                    offload_plane(oa, t_wa2, ta, p)
                psA = psum.tile([128, NF * W], f32, tag="psA")
                conv_mms(psA, t_wa, ta, 128, 0, NF)
                copy_half(oa, psA, 0, alt=(g % 2 == 0), npl=NF)
            else:
                for h in range(O // QP):
                    psA = psum.tile([128, QP * W], f32, tag="psA")
                    conv_mms(psA, t_wa, ta, 128, h * QP)
                    copy_half(oa, psA, h * QP * W, alt=((g + h) % 2 == 0))
            nc.scalar.dma_start(out=d_out[g, 0:128, :], in_=oa[:])

            ob = outp.tile([128, O * W], f16, tag="ob")
            if offload:
                for p in range(NF, O):
                    offload_plane(ob, t_wb2, tb, p)
                psB = psum.tile([128, NF * W], f32, tag="psA")
                conv_mms(psB, t_wb, tb, 128, 0, NF)
                copy_half(ob, psB, 0, alt=(g % 2 == 1), npl=NF)
            else:
                for h in range(O // QP):
                    psB = psum.tile([128, QP * W], f32, tag="psA")
                    conv_mms(psB, t_wb, tb, 128, h * QP)
                    copy_half(ob, psB, h * QP * W, alt=((g + h) % 2 == 1))
            nc.scalar.dma_start(out=d_out[g, 128:256, :], in_=ob[:])

            # stacked remainder: input rows come straight from DRAM, so
            # emit early (oct 2, 4, ...) to keep them off the kernel tail
            if ri < len(rem_groups) and g == min(2 * (ri + 1), noct - 1):
                s, gsz = rem_groups[ri]
                ri += 1
                tr = xinr.tile([RG * 6, O * W], f16, tag="tr")
                nc.sync.dma_start(
                    out=tr[: 6 * gsz, :], in_=d_xs[s : s + gsz, 250:256, :]
                )
                orr = outr.tile([RG * 4, O * W], f16, tag="orr")
                for h in range(O // QP):
                    psR = psum.tile([RG * 4, QP * W], f32, tag="psA")
                    last = (SHIFT_ORDER[-1], QP - 1)
                    for i in SHIFT_ORDER:
                        wl, wh, cl, ch = SHIFT_RANGES[i]
                        lhsT = t_wr[: 6 * gsz, i * RG * 4 : i * RG * 4 + 4 * gsz]
                        for q in range(QP):
                            nc.tensor.matmul(
                                psR[: 4 * gsz, q * W + wl : q * W + wh],
                                lhsT,
                                tr[: 6 * gsz, (h * QP + q) * W + cl : (h * QP + q) * W + ch],
                                start=(i == SHIFT_ORDER[0] and q % 2 == 0),
                                stop=((i, q) == last),
                                skip_group_check=True,
                            )
                    if (g + h) % 2 == 0:
                        nc.scalar.copy(
                            orr[: 4 * gsz, h * QP * W : (h + 1) * QP * W],
                            psR[: 4 * gsz, :],
                        )
                    else:
                        nc.vector.tensor_copy(
                            orr[: 4 * gsz, h * QP * W : (h + 1) * QP * W],
                            psR[: 4 * gsz, :],
                        )
                nc.scalar.dma_start(
                    out=d_out[s : s + gsz, H : H + 4, :], in_=orr[: 4 * gsz, :]
                )

    nc.compile()
    return nc


_CACHE = {}


def _get_program(noct: int = NOCT, ratios=None):
    key = (noct, ratios)
    if key not in _CACHE:
        _CACHE[key] = _build_program(noct, ratios)
    return _CACHE[key]


def _run(x: np.ndarray, wk: np.ndarray, trace: bool = False):
    """x: [P, 256, 256] f32 full stack of planes (P divisible by 8*O),
    wk: flipped 4x4 kernel. Returns ([P, 256, 256] f32, exec_time_ns|None)."""
    P = x.shape[0]
    oper = P // (N_CORES * O)
    hi = x.astype(np.float16)
    # oct-pack: [P/O, O, H, W] -> [P/O, H, O, W] -> [P/O, H, O*W]
    xso = (
        hi.reshape(P // O, O, H, W)
        .transpose(0, 2, 1, 3)
        .reshape(P // O, H, O * W)
    )

    wa, wb, wr, wa2, wb2 = _make_weights(wk)
    sep = _separable(wk)
    ratios = None
    if sep is not None:
        uh, uw = sep
        ratios = (
            float(uw[0] / uw[1]),
            float(uw[3] / uw[2]),
            float(uw[1] / uw[2]),
        )
    nc = _get_program(oper, ratios)

    in_maps = [
        {
            "xs": np.ascontiguousarray(xso[c * oper : (c + 1) * oper]),
            "wa": wa,
            "wb": wb,
            "wr": wr,
            "wa2": wa2,
            "wb2": wb2,
        }
        for c in range(N_CORES)
    ]
    res = run_bass_kernel_spmd(nc, in_maps, list(range(N_CORES)), trace=trace)
    outq = np.concatenate([r["out"] for r in res.results], axis=0)  # [P/O, H+4, O*W]
    outq = np.concatenate(
        [outq[:, 0:127], outq[:, 128:253], outq[:, 256:260]], axis=1
    )  # drop junk rows -> [P/O, 256, O*W]
    out = (
        outq.reshape(P // O, H, O, W)
        .transpose(0, 2, 1, 3)
        .reshape(P, H, W)
        .astype(np.float32)
    )
    return np.ascontiguousarray(out), res.exec_time_ns


def kernel(input: np.ndarray, kernel: np.ndarray) -> np.ndarray:
    x = np.asarray(input, dtype=np.float32)
    k = np.asarray(kernel, dtype=np.float32)
    n, c, h, w = x.shape
    wk = np.flip(k, (0, 1)).copy()  # correlation weights
    out, _ = _run(x.reshape(n * c, h, w), wk, trace=False)
    return out.reshape(n, c, h, w)
